# revision 4
# baseline (speedup 1.0000x reference)
"""ActiveDendriteLayer on 8 TRN2 NeuronCores.

reference:
    h = relu(x @ W_dend.T + b_dend)          # [B, 12288]
    h = h.reshape(B, 4096, 3)
    out = einsum('bcd,cd->bc', h, W_local) + b_local   # [B, 4096]

Sharding: tensor-parallel over cells. Core i owns cells [i*512, (i+1)*512)
= dendrite rows [i*1536, (i+1)*1536). Each core reads the full x (bf16,
17 MB), keeps its W_dend shard resident in SBUF (6.7 MB bf16), and writes
out[:, i*512:(i+1)*512].

Host-side prep (inside kernel(), outside the NEFF):
- dendrite rows permuted d-major (j' = d*512 + c) so the on-chip combine
  uses three contiguous 512-column slices instead of stride-3 reads;
- b_dend folded into the matmul via an extra contraction row of ones
  (K = 2048+1, zero-padded to 17*128 = 2176);
- x and W shards pre-transposed/pre-tiled into exact SBUF images so every
  DMA is a fully contiguous per-partition copy;
- x / W cast to bf16 (PE runs fp32 matmul at 1/4 rate; PSUM still
  accumulates fp32).

Device kernel per core, per b-tile (128 batch rows):
  17 K-chunks x 3 (N=512) matmuls accumulate h^T-image [128b, 1536j'] in
  PSUM (3 banks); DVE epilogue: out = relu(h_d)*wl_d summed over d=0..2
  plus b_local, via fused scalar_tensor_tensor(max 0, mult) ops; DMA out.
"""

import sys

for _p in ("/opt/trn_rl_repo",):
    if _p not in sys.path:
        sys.path.append(_p)

import ml_dtypes
import numpy as np

import concourse.bass as bass
import concourse.tile as tile
from concourse import mybir
from concourse.bass_utils import run_bass_kernel_spmd

B, D = 4096, 2048
N_CELLS, N_DEND = 4096, 3
N_CORES = 8
C = N_CELLS // N_CORES          # 512 cells per core
J = C * N_DEND                  # 1536 dendrites per core
KP = 128                        # contraction rows per chunk
KC = 17                         # chunks: 16*128 for D + 1 for the bias row
K_AUG = KC * KP                 # 2176 (row 2048 = ones/b_dend, rest zero pad)
BT = 32                         # batch tiles
BTP = 128                       # batch rows per tile
NJ = 512                        # matmul free dim = one PSUM bank

BF16 = ml_dtypes.bfloat16


def _split_multi_waits(nc: bass.Bass) -> None:
    """Walrus in this container enforces the cayman ISA's one-sync-wait-
    per-instruction encoding ("Too many sync wait commands") instead of
    splitting them itself. Hoist extra waits onto same-engine NOPs placed
    immediately before the instruction (engines execute in order, so the
    waits still all complete before the instruction issues)."""
    idx = 0
    for f in nc.m.functions:
        for bb in f.blocks:
            new: list = []
            for inst in bb.instructions:
                si = inst.sync_info
                if si is not None and si.on_update and len(si.on_update) > 1:
                    raise RuntimeError(
                        f"{inst.name}: {len(si.on_update)} sync updates; "
                        "walrus supports 1 and updates can't be hoisted")
                if si is not None and si.on_wait and len(si.on_wait) > 1:
                    waits = list(si.on_wait)
                    for w in waits[:-1]:
                        nop = mybir.InstNoOp(
                            name=f"mwsplit_{idx}", ins=[], outs=[])
                        idx += 1
                        nop.engine = inst.engine
                        nop.sync_info = mybir.SyncInfo(
                            on_wait=[w], on_update=[])
                        new.append(nop)
                    inst.sync_info = mybir.SyncInfo(
                        on_wait=[waits[-1]], on_update=list(si.on_update))
                new.append(inst)
            bb.instructions[:] = new


def build_kernel() -> bass.Bass:
    nc = bass.Bass("TRN2", target_bir_lowering=False, debug=False,
                   num_devices=N_CORES)
    ximg_ext = nc.declare_dram_parameter(
        "ximg", [BT, BTP, KC * BTP], mybir.dt.bfloat16, isOutput=False)
    wimg_ext = nc.declare_dram_parameter(
        "wimg", [KP, KC * J], mybir.dt.bfloat16, isOutput=False)
    wl_ext = nc.declare_dram_parameter(
        "wl", [128, 4 * C], mybir.dt.float32, isOutput=False)
    out_ext = nc.declare_dram_parameter(
        "out", [B, C], mybir.dt.float32, isOutput=True)

    AT = mybir.AluOpType
    with tile.TileContext(nc) as tc:
        with (
            tc.tile_pool(name="wres", bufs=1) as wres,
            tc.tile_pool(name="xin", bufs=3) as xin,
            tc.tile_pool(name="oeps", bufs=2) as oeps,
            tc.tile_pool(name="ps", bufs=2, space=bass.MemorySpace.PSUM) as psp,
        ):
            wt = wres.tile([KP, KC * J], mybir.dt.bfloat16)
            # split the 6.7MB resident-W load so the first K-chunk's
            # matmuls can start after ~1/17th of it has landed
            for kc in range(KC):
                nc.sync.dma_start(
                    wt[:, kc * J:(kc + 1) * J],
                    wimg_ext[:, kc * J:(kc + 1) * J])
            wl = wres.tile([128, 4 * C], mybir.dt.float32)
            nc.sync.dma_start(wl[:], wl_ext[:])

            for bt in range(BT):
                xt = xin.tile([BTP, KC * BTP], mybir.dt.bfloat16)
                nc.sync.dma_start(xt[:], ximg_ext[bt])

                ps = psp.tile([BTP, J], mybir.dt.float32)
                for kc in range(KC):
                    lhsT = xt[:, kc * BTP:(kc + 1) * BTP]
                    for jc in range(3):
                        nc.tensor.matmul(
                            ps[:, jc * NJ:(jc + 1) * NJ],
                            lhsT,
                            wt[:, kc * J + jc * NJ: kc * J + (jc + 1) * NJ],
                            start=(kc == 0),
                            stop=(kc == KC - 1),
                        )

                acc = oeps.tile([BTP, C], mybir.dt.float32)
                tmp = oeps.tile([BTP, C], mybir.dt.float32)
                # acc = relu(h_d0) * wl_d0 ; += relu(h_d1) * wl_d1 ; ...
                nc.vector.scalar_tensor_tensor(
                    acc[:], ps[:, 0:NJ], 0.0, wl[:, 0:C],
                    op0=AT.max, op1=AT.mult)
                nc.vector.scalar_tensor_tensor(
                    tmp[:], ps[:, NJ:2 * NJ], 0.0, wl[:, C:2 * C],
                    op0=AT.max, op1=AT.mult)
                nc.vector.tensor_add(acc[:], acc[:], tmp[:])
                nc.vector.scalar_tensor_tensor(
                    tmp[:], ps[:, 2 * NJ:3 * NJ], 0.0, wl[:, 2 * C:3 * C],
                    op0=AT.max, op1=AT.mult)
                nc.vector.tensor_add(acc[:], acc[:], tmp[:])
                nc.vector.tensor_add(acc[:], acc[:], wl[:, 3 * C:4 * C])

                nc.sync.dma_start(out_ext[bt * BTP:(bt + 1) * BTP, :], acc[:])

    _split_multi_waits(nc)
    return nc


def _host_images(x, W_dend, b_dend, W_local, b_local):
    # shared x image: [bt, p(k-in-chunk), kc*128 + bb], bf16
    x_aug = np.zeros((B, K_AUG), np.float32)
    x_aug[:, :D] = x
    x_aug[:, D] = 1.0
    ximg = np.ascontiguousarray(
        x_aug.reshape(BT, BTP, KC, KP).transpose(0, 3, 2, 1)
    ).reshape(BT, KP, KC * BTP).astype(BF16)

    wimgs, wls = [], []
    for i in range(N_CORES):
        sl = slice(i * J, (i + 1) * J)
        # d-major dendrite permutation: row j' = d*C + c <- shard row c*3+d
        W_dm = np.ascontiguousarray(
            W_dend[sl].reshape(C, N_DEND, D).transpose(1, 0, 2)
        ).reshape(J, D)
        b_dm = np.ascontiguousarray(
            b_dend[sl].reshape(C, N_DEND).T).reshape(J)
        Wt_aug = np.zeros((K_AUG, J), np.float32)
        Wt_aug[:D] = W_dm.T
        Wt_aug[D] = b_dm
        wimg = np.ascontiguousarray(
            Wt_aug.reshape(KC, KP, J).transpose(1, 0, 2)
        ).reshape(KP, KC * J).astype(BF16)
        wimgs.append(wimg)

        wlc = W_local[i * C:(i + 1) * C]          # [C, 3]
        blc = b_local[i * C:(i + 1) * C]          # [C]
        wl = np.empty((128, 4 * C), np.float32)
        wl[:, 0 * C:1 * C] = wlc[:, 0][None, :]
        wl[:, 1 * C:2 * C] = wlc[:, 1][None, :]
        wl[:, 2 * C:3 * C] = wlc[:, 2][None, :]
        wl[:, 3 * C:4 * C] = blc[None, :]
        wls.append(wl)
    return ximg, wimgs, wls


_RUN_KWARGS = {}


def kernel(x, W_dend, b_dend, W_local, b_local):
    x = np.asarray(x, np.float32)
    W_dend = np.asarray(W_dend, np.float32)
    b_dend = np.asarray(b_dend, np.float32)
    W_local = np.asarray(W_local, np.float32)
    b_local = np.asarray(b_local, np.float32)

    ximg, wimgs, wls = _host_images(x, W_dend, b_dend, W_local, b_local)
    nc = build_kernel()
    in_maps = [
        {"ximg": ximg, "wimg": wimgs[i], "wl": wls[i]}
        for i in range(N_CORES)
    ]
    res = run_bass_kernel_spmd(
        nc, in_maps, core_ids=list(range(N_CORES)), **_RUN_KWARGS)
    out = np.concatenate(
        [np.asarray(res.results[i]["out"], np.float32)
         for i in range(N_CORES)],
        axis=1,
    )
    kernel.last_results = res
    return out


# revision 9
# speedup vs baseline: 1.2122x; 1.2122x over previous
"""ActiveDendriteLayer on 8 TRN2 NeuronCores.

reference:
    h = relu(x @ W_dend.T + b_dend)          # [B, 12288]
    h = h.reshape(B, 4096, 3)
    out = einsum('bcd,cd->bc', h, W_local) + b_local   # [B, 4096]

Sharding: tensor-parallel over cells. Core i owns cells [i*512, (i+1)*512)
= dendrite rows [i*1536, (i+1)*1536). Each core reads the full x (bf16,
17 MB), keeps its W_dend shard resident in SBUF (6.7 MB bf16), and writes
out[:, i*512:(i+1)*512].

Host-side prep (inside kernel(), outside the NEFF):
- dendrite rows permuted d-major (j' = d*512 + c) so the on-chip combine
  uses three contiguous 512-column slices instead of stride-3 reads;
- b_dend folded into the matmul via an extra contraction row of ones
  (K = 2048+1, zero-padded to 17*128 = 2176);
- x and W shards pre-transposed/pre-tiled into exact SBUF images so every
  DMA is a fully contiguous per-partition copy;
- x / W cast to bf16 (PE runs fp32 matmul at 1/4 rate; PSUM still
  accumulates fp32).

Device kernel per core, per b-tile (128 batch rows):
  17 K-chunks x 3 (N=512) matmuls accumulate h^T-image [128b, 1536j'] in
  PSUM (3 banks); DVE epilogue: out = relu(h_d)*wl_d summed over d=0..2
  plus b_local, via fused scalar_tensor_tensor(max 0, mult) ops; DMA out.
"""

import sys

for _p in ("/opt/trn_rl_repo",):
    if _p not in sys.path:
        sys.path.append(_p)

import ml_dtypes
import numpy as np

import concourse.bass as bass
import concourse.tile as tile
from concourse import mybir
from concourse.bass_utils import run_bass_kernel_spmd

B, D = 4096, 2048
N_CELLS, N_DEND = 4096, 3
N_CORES = 8
C = N_CELLS // N_CORES          # 512 cells per core
J = C * N_DEND                  # 1536 dendrites per core
KP = 128                        # contraction rows per chunk
KC = 17                         # chunks: 16*128 for D + 1 for the bias row
K_AUG = KC * KP                 # 2176 (row 2048 = ones/b_dend, rest zero pad)
BT = 32                         # batch tiles
BTP = 128                       # batch rows per tile
NJ = 512                        # matmul free dim = one PSUM bank

BF16 = ml_dtypes.bfloat16


def _dedup_ldweights(nc: bass.Bass) -> None:
    """Tile lowers every matmul to InstLdweights + InstMatmult. Our inner
    loop issues 3 matmuls per stationary x-chunk (one per PSUM j-slice),
    so 2 of 3 weight loads are redundant; each costs ~45 ns of serialized
    PE time (walrus here has no LDW dedup for explicit InstLdweights).
    Drop an InstLdweights when it loads the exact AP the PE already holds
    and only InstMatmults ran on PE since. Carried sync waits are moved to
    the next PE instruction (the split pass then hoists extras to NOPs)."""
    for f in nc.m.functions:
        for bb in f.blocks:
            new: list = []
            last_sig = None
            pending_waits: list = []
            for inst in bb.instructions:
                tn = type(inst).__name__
                if getattr(inst, "engine", None) != mybir.EngineType.PE:
                    new.append(inst)
                    continue
                if tn == "InstLdweights":
                    sig = str(inst.ins[0])
                    si = inst.sync_info
                    if sig == last_sig and not (si and si.on_update):
                        if si and si.on_wait:
                            pending_waits.extend(si.on_wait)
                        continue  # drop redundant load
                    last_sig = sig
                elif tn != "InstMatmult":
                    last_sig = None  # any other PE inst may clobber state
                if pending_waits:
                    si = inst.sync_info
                    waits = list(si.on_wait) if si and si.on_wait else []
                    ups = list(si.on_update) if si and si.on_update else []
                    inst.sync_info = mybir.SyncInfo(
                        on_wait=pending_waits + waits, on_update=ups)
                    pending_waits = []
                new.append(inst)
            assert not pending_waits
            bb.instructions[:] = new


def _split_multi_waits(nc: bass.Bass) -> None:
    """Walrus in this container enforces the cayman ISA's one-sync-wait-
    per-instruction encoding ("Too many sync wait commands") instead of
    splitting them itself. Hoist extra waits onto same-engine NOPs placed
    immediately before the instruction (engines execute in order, so the
    waits still all complete before the instruction issues)."""
    idx = 0
    for f in nc.m.functions:
        for bb in f.blocks:
            new: list = []
            for inst in bb.instructions:
                si = inst.sync_info
                if si is not None and si.on_update and len(si.on_update) > 1:
                    raise RuntimeError(
                        f"{inst.name}: {len(si.on_update)} sync updates; "
                        "walrus supports 1 and updates can't be hoisted")
                if si is not None and si.on_wait and len(si.on_wait) > 1:
                    waits = list(si.on_wait)
                    for w in waits[:-1]:
                        nop = mybir.InstNoOp(
                            name=f"mwsplit_{idx}", ins=[], outs=[])
                        idx += 1
                        nop.engine = inst.engine
                        nop.sync_info = mybir.SyncInfo(
                            on_wait=[w], on_update=[])
                        new.append(nop)
                    inst.sync_info = mybir.SyncInfo(
                        on_wait=[waits[-1]], on_update=list(si.on_update))
                new.append(inst)
            bb.instructions[:] = new


def build_kernel() -> bass.Bass:
    nc = bass.Bass("TRN2", target_bir_lowering=False, debug=False,
                   num_devices=N_CORES)
    ximg_ext = nc.declare_dram_parameter(
        "ximg", [BT, BTP, KC * BTP], mybir.dt.bfloat16, isOutput=False)
    wimg_ext = nc.declare_dram_parameter(
        "wimg", [KP, KC * J], mybir.dt.bfloat16, isOutput=False)
    wl_ext = nc.declare_dram_parameter(
        "wl", [128, 4 * C], mybir.dt.float32, isOutput=False)
    out_ext = nc.declare_dram_parameter(
        "out", [B, C], mybir.dt.float32, isOutput=True)

    AT = mybir.AluOpType
    with tile.TileContext(nc) as tc:
        with (
            tc.tile_pool(name="wres", bufs=1) as wres,
            tc.tile_pool(name="xin", bufs=3) as xin,
            tc.tile_pool(name="oeps", bufs=2) as oeps,
            tc.tile_pool(name="ps", bufs=2, space=bass.MemorySpace.PSUM) as psp,
        ):
            wt = wres.tile([KP, KC * J], mybir.dt.bfloat16)
            wl = wres.tile([128, 4 * C], mybir.dt.float32)
            # first x tile before the 6.7MB resident-W load so the first
            # matmul isn't queued behind it; W split per K-chunk so kc=0
            # can start after ~1/17th of it has landed
            xt0 = xin.tile([BTP, KC * BTP], mybir.dt.bfloat16, name="xt")
            nc.sync.dma_start(xt0[:], ximg_ext[0])
            for kc in range(KC):
                nc.sync.dma_start(
                    wt[:, kc * J:(kc + 1) * J],
                    wimg_ext[:, kc * J:(kc + 1) * J])
            nc.sync.dma_start(wl[:], wl_ext[:])

            for bt in range(BT):
                if bt == 0:
                    xt = xt0
                else:
                    xt = xin.tile([BTP, KC * BTP], mybir.dt.bfloat16,
                                  name="xt")
                    nc.sync.dma_start(xt[:], ximg_ext[bt])

                ps = psp.tile([BTP, J], mybir.dt.float32)
                for kc in range(KC):
                    lhsT = xt[:, kc * BTP:(kc + 1) * BTP]
                    for jc in range(3):
                        nc.tensor.matmul(
                            ps[:, jc * NJ:(jc + 1) * NJ],
                            lhsT,
                            wt[:, kc * J + jc * NJ: kc * J + (jc + 1) * NJ],
                            start=(kc == 0),
                            stop=(kc == KC - 1),
                        )

                acc = oeps.tile([BTP, C], mybir.dt.float32)
                tmp = oeps.tile([BTP, C], mybir.dt.float32)
                # acc = relu(h_d0) * wl_d0 ; += relu(h_d1) * wl_d1 ; ...
                nc.vector.scalar_tensor_tensor(
                    acc[:], ps[:, 0:NJ], 0.0, wl[:, 0:C],
                    op0=AT.max, op1=AT.mult)
                nc.vector.scalar_tensor_tensor(
                    tmp[:], ps[:, NJ:2 * NJ], 0.0, wl[:, C:2 * C],
                    op0=AT.max, op1=AT.mult)
                nc.vector.tensor_add(acc[:], acc[:], tmp[:])
                nc.vector.scalar_tensor_tensor(
                    tmp[:], ps[:, 2 * NJ:3 * NJ], 0.0, wl[:, 2 * C:3 * C],
                    op0=AT.max, op1=AT.mult)
                nc.vector.tensor_add(acc[:], acc[:], tmp[:])
                nc.vector.tensor_add(acc[:], acc[:], wl[:, 3 * C:4 * C])

                nc.sync.dma_start(out_ext[bt * BTP:(bt + 1) * BTP, :], acc[:])

    _dedup_ldweights(nc)
    _split_multi_waits(nc)
    return nc


def _host_images(x, W_dend, b_dend, W_local, b_local):
    # shared x image: [bt, p(k-in-chunk), kc*128 + bb], bf16
    x_aug = np.zeros((B, K_AUG), np.float32)
    x_aug[:, :D] = x
    x_aug[:, D] = 1.0
    ximg = np.ascontiguousarray(
        x_aug.reshape(BT, BTP, KC, KP).transpose(0, 3, 2, 1)
    ).reshape(BT, KP, KC * BTP).astype(BF16)

    wimgs, wls = [], []
    for i in range(N_CORES):
        sl = slice(i * J, (i + 1) * J)
        # d-major dendrite permutation: row j' = d*C + c <- shard row c*3+d
        W_dm = np.ascontiguousarray(
            W_dend[sl].reshape(C, N_DEND, D).transpose(1, 0, 2)
        ).reshape(J, D)
        b_dm = np.ascontiguousarray(
            b_dend[sl].reshape(C, N_DEND).T).reshape(J)
        Wt_aug = np.zeros((K_AUG, J), np.float32)
        Wt_aug[:D] = W_dm.T
        Wt_aug[D] = b_dm
        wimg = np.ascontiguousarray(
            Wt_aug.reshape(KC, KP, J).transpose(1, 0, 2)
        ).reshape(KP, KC * J).astype(BF16)
        wimgs.append(wimg)

        wlc = W_local[i * C:(i + 1) * C]          # [C, 3]
        blc = b_local[i * C:(i + 1) * C]          # [C]
        wl = np.empty((128, 4 * C), np.float32)
        wl[:, 0 * C:1 * C] = wlc[:, 0][None, :]
        wl[:, 1 * C:2 * C] = wlc[:, 1][None, :]
        wl[:, 2 * C:3 * C] = wlc[:, 2][None, :]
        wl[:, 3 * C:4 * C] = blc[None, :]
        wls.append(wl)
    return ximg, wimgs, wls


_RUN_KWARGS = {}


def kernel(x, W_dend, b_dend, W_local, b_local):
    x = np.asarray(x, np.float32)
    W_dend = np.asarray(W_dend, np.float32)
    b_dend = np.asarray(b_dend, np.float32)
    W_local = np.asarray(W_local, np.float32)
    b_local = np.asarray(b_local, np.float32)

    ximg, wimgs, wls = _host_images(x, W_dend, b_dend, W_local, b_local)
    nc = build_kernel()
    in_maps = [
        {"ximg": ximg, "wimg": wimgs[i], "wl": wls[i]}
        for i in range(N_CORES)
    ]
    res = run_bass_kernel_spmd(
        nc, in_maps, core_ids=list(range(N_CORES)), **_RUN_KWARGS)
    out = np.concatenate(
        [np.asarray(res.results[i]["out"], np.float32)
         for i in range(N_CORES)],
        axis=1,
    )
    kernel.last_results = res
    return out


# revision 10
# speedup vs baseline: 1.3039x; 1.0757x over previous
"""ActiveDendriteLayer on 8 TRN2 NeuronCores.

reference:
    h = relu(x @ W_dend.T + b_dend)          # [B, 12288]
    h = h.reshape(B, 4096, 3)
    out = einsum('bcd,cd->bc', h, W_local) + b_local   # [B, 4096]

Sharding: tensor-parallel over cells. Core i owns cells [i*512, (i+1)*512)
= dendrite rows [i*1536, (i+1)*1536). Each core reads the full x (bf16,
17 MB), keeps its W_dend shard resident in SBUF (6.3 MB bf16), and writes
out[:, i*512:(i+1)*512].

Host-side prep (inside kernel(), outside the NEFF):
- dendrite rows permuted d-major (j' = d*512 + c) so the on-chip combine
  uses three contiguous 512-column slices instead of stride-3 reads;
- x and W shards pre-transposed/pre-tiled into exact SBUF images so every
  DMA is a fully contiguous per-partition copy;
- x / W cast to bf16 (PE runs fp32 matmul at 1/4 rate; PSUM still
  accumulates fp32). b_dend/W_local/b_local stay fp32.

Device kernel per core, per b-tile (128 batch rows):
  16 K-chunks x 3 (N=512) matmuls accumulate h^T-image [128b, 1536j'] in
  PSUM (3 banks, double-buffered); DVE epilogue: hb = psum + b_dend, then
  out = sum_d relu(hb_d)*wl_d + b_local via fused
  scalar_tensor_tensor(max 0, mult) ops; DMA out. PE is the bottleneck
  (~216 ns per N=512 bf16 matmul, LDWEIGHTS hidden by the dedup pass).
"""

import sys

for _p in ("/opt/trn_rl_repo",):
    if _p not in sys.path:
        sys.path.append(_p)

import ml_dtypes
import numpy as np

import concourse.bass as bass
import concourse.tile as tile
from concourse import mybir
from concourse.bass_utils import run_bass_kernel_spmd

B, D = 4096, 2048
N_CELLS, N_DEND = 4096, 3
N_CORES = 8
C = N_CELLS // N_CORES          # 512 cells per core
J = C * N_DEND                  # 1536 dendrites per core
KP = 128                        # contraction rows per chunk
KC = D // KP                    # 16 K-chunks
BT = 32                         # batch tiles
BTP = 128                       # batch rows per tile
NJ = 512                        # matmul free dim = one PSUM bank

BF16 = ml_dtypes.bfloat16


def _dedup_ldweights(nc: bass.Bass) -> None:
    """Tile lowers every matmul to InstLdweights + InstMatmult. Our inner
    loop issues 3 matmuls per stationary x-chunk (one per PSUM j-slice),
    so 2 of 3 weight loads are redundant; each costs ~45 ns of serialized
    PE time (walrus here has no LDW dedup for explicit InstLdweights).
    Drop an InstLdweights when it loads the exact AP the PE already holds
    and only InstMatmults ran on PE since. Carried sync waits are moved to
    the next PE instruction (the split pass then hoists extras to NOPs)."""
    for f in nc.m.functions:
        for bb in f.blocks:
            new: list = []
            last_sig = None
            pending_waits: list = []
            for inst in bb.instructions:
                tn = type(inst).__name__
                if getattr(inst, "engine", None) != mybir.EngineType.PE:
                    new.append(inst)
                    continue
                if tn == "InstLdweights":
                    sig = str(inst.ins[0])
                    si = inst.sync_info
                    if sig == last_sig and not (si and si.on_update):
                        if si and si.on_wait:
                            pending_waits.extend(si.on_wait)
                        continue  # drop redundant load
                    last_sig = sig
                elif tn != "InstMatmult":
                    last_sig = None  # any other PE inst may clobber state
                if pending_waits:
                    si = inst.sync_info
                    waits = list(si.on_wait) if si and si.on_wait else []
                    ups = list(si.on_update) if si and si.on_update else []
                    inst.sync_info = mybir.SyncInfo(
                        on_wait=pending_waits + waits, on_update=ups)
                    pending_waits = []
                new.append(inst)
            assert not pending_waits
            bb.instructions[:] = new


def _split_multi_waits(nc: bass.Bass) -> None:
    """Walrus in this container enforces the cayman ISA's one-sync-wait-
    per-instruction encoding ("Too many sync wait commands") instead of
    splitting them itself. Hoist extra waits onto same-engine NOPs placed
    immediately before the instruction (engines execute in order, so the
    waits still all complete before the instruction issues)."""
    idx = 0
    for f in nc.m.functions:
        for bb in f.blocks:
            new: list = []
            for inst in bb.instructions:
                si = inst.sync_info
                if si is not None and si.on_update and len(si.on_update) > 1:
                    raise RuntimeError(
                        f"{inst.name}: {len(si.on_update)} sync updates; "
                        "walrus supports 1 and updates can't be hoisted")
                if si is not None and si.on_wait and len(si.on_wait) > 1:
                    waits = list(si.on_wait)
                    for w in waits[:-1]:
                        nop = mybir.InstNoOp(
                            name=f"mwsplit_{idx}", ins=[], outs=[])
                        idx += 1
                        nop.engine = inst.engine
                        nop.sync_info = mybir.SyncInfo(
                            on_wait=[w], on_update=[])
                        new.append(nop)
                    inst.sync_info = mybir.SyncInfo(
                        on_wait=[waits[-1]], on_update=list(si.on_update))
                new.append(inst)
            bb.instructions[:] = new


def build_kernel() -> bass.Bass:
    nc = bass.Bass("TRN2", target_bir_lowering=False, debug=False,
                   num_devices=N_CORES)
    ximg_ext = nc.declare_dram_parameter(
        "ximg", [BT, BTP, KC * BTP], mybir.dt.bfloat16, isOutput=False)
    wimg_ext = nc.declare_dram_parameter(
        "wimg", [KP, KC * J], mybir.dt.bfloat16, isOutput=False)
    # fp32 epilogue constants, one row-image: bd (b_dend, d-major, 1536)
    # | wl0|wl1|wl2 (512 each) | bl (512)  -> [128, 3584]
    wl_ext = nc.declare_dram_parameter(
        "wl", [128, J + 4 * C], mybir.dt.float32, isOutput=False)
    out_ext = nc.declare_dram_parameter(
        "out", [B, C], mybir.dt.float32, isOutput=True)

    AT = mybir.AluOpType
    with tile.TileContext(nc) as tc:
        with (
            tc.tile_pool(name="wres", bufs=1) as wres,
            tc.tile_pool(name="xin", bufs=4) as xin,
            tc.tile_pool(name="eps", bufs=2) as eps,
            tc.tile_pool(name="ps", bufs=2, space=bass.MemorySpace.PSUM) as psp,
        ):
            wt = wres.tile([KP, KC * J], mybir.dt.bfloat16)
            wl = wres.tile([128, J + 4 * C], mybir.dt.float32)

            # Early-DMA order: first x tiles interleaved between resident-W
            # chunk loads so b-tiles 0..3 are fed while W streams in.
            xts = [None] * BT
            xts[0] = xin.tile([BTP, KC * BTP], mybir.dt.bfloat16, name="xt")
            nc.sync.dma_start(xts[0][:], ximg_ext[0])
            prefetch_at = {4: 1, 8: 2, 12: 3}
            for kc in range(KC):
                nc.sync.dma_start(
                    wt[:, kc * J:(kc + 1) * J],
                    wimg_ext[:, kc * J:(kc + 1) * J])
                nb = prefetch_at.get(kc)
                if nb is not None:
                    xts[nb] = xin.tile(
                        [BTP, KC * BTP], mybir.dt.bfloat16, name="xt")
                    nc.sync.dma_start(xts[nb][:], ximg_ext[nb])
            nc.sync.dma_start(wl[:], wl_ext[:])

            bd = wl[:, 0:J]
            wld = [wl[:, J + d * C: J + (d + 1) * C] for d in range(3)]
            blb = wl[:, J + 3 * C: J + 4 * C]

            for bt in range(BT):
                xt = xts[bt]
                if xt is None:
                    xt = xin.tile([BTP, KC * BTP], mybir.dt.bfloat16,
                                  name="xt")
                    nc.sync.dma_start(xt[:], ximg_ext[bt])

                ps = psp.tile([BTP, J], mybir.dt.float32)
                for kc in range(KC):
                    lhsT = xt[:, kc * BTP:(kc + 1) * BTP]
                    for jc in range(3):
                        nc.tensor.matmul(
                            ps[:, jc * NJ:(jc + 1) * NJ],
                            lhsT,
                            wt[:, kc * J + jc * NJ: kc * J + (jc + 1) * NJ],
                            start=(kc == 0),
                            stop=(kc == KC - 1),
                        )

                hb = eps.tile([BTP, J], mybir.dt.float32)
                acc = eps.tile([BTP, C], mybir.dt.float32)
                tmp = eps.tile([BTP, C], mybir.dt.float32)
                # hb = h + b_dend ; acc = sum_d relu(hb_d)*wl_d ; + b_local
                nc.vector.tensor_add(hb[:], ps[:, :], bd)
                nc.vector.scalar_tensor_tensor(
                    acc[:], hb[:, 0:C], 0.0, wld[0],
                    op0=AT.max, op1=AT.mult)
                nc.vector.scalar_tensor_tensor(
                    tmp[:], hb[:, C:2 * C], 0.0, wld[1],
                    op0=AT.max, op1=AT.mult)
                nc.vector.tensor_add(acc[:], acc[:], tmp[:])
                nc.vector.scalar_tensor_tensor(
                    tmp[:], hb[:, 2 * C:3 * C], 0.0, wld[2],
                    op0=AT.max, op1=AT.mult)
                nc.vector.tensor_add(acc[:], acc[:], tmp[:])
                nc.vector.tensor_add(acc[:], acc[:], blb)

                nc.sync.dma_start(out_ext[bt * BTP:(bt + 1) * BTP, :], acc[:])

    _dedup_ldweights(nc)
    _split_multi_waits(nc)
    return nc


def _host_images(x, W_dend, b_dend, W_local, b_local):
    # shared x image: [bt, p(k-in-chunk), kc*128 + bb], bf16
    ximg = np.ascontiguousarray(
        x.reshape(BT, BTP, KC, KP).transpose(0, 3, 2, 1)
    ).reshape(BT, KP, KC * BTP).astype(BF16)

    wimgs, wls = [], []
    for i in range(N_CORES):
        sl = slice(i * J, (i + 1) * J)
        # d-major dendrite permutation: row j' = d*C + c <- shard row c*3+d
        W_dm = np.ascontiguousarray(
            W_dend[sl].reshape(C, N_DEND, D).transpose(1, 0, 2)
        ).reshape(J, D)
        b_dm = np.ascontiguousarray(
            b_dend[sl].reshape(C, N_DEND).T).reshape(J)
        wimg = np.ascontiguousarray(
            W_dm.T.reshape(KC, KP, J).transpose(1, 0, 2)
        ).reshape(KP, KC * J).astype(BF16)
        wimgs.append(wimg)

        wlc = W_local[i * C:(i + 1) * C]          # [C, 3]
        blc = b_local[i * C:(i + 1) * C]          # [C]
        wl = np.empty((128, J + 4 * C), np.float32)
        wl[:, 0:J] = b_dm[None, :]
        wl[:, J + 0 * C:J + 1 * C] = wlc[:, 0][None, :]
        wl[:, J + 1 * C:J + 2 * C] = wlc[:, 1][None, :]
        wl[:, J + 2 * C:J + 3 * C] = wlc[:, 2][None, :]
        wl[:, J + 3 * C:J + 4 * C] = blc[None, :]
        wls.append(wl)
    return ximg, wimgs, wls


_RUN_KWARGS = {}


def kernel(x, W_dend, b_dend, W_local, b_local):
    x = np.asarray(x, np.float32)
    W_dend = np.asarray(W_dend, np.float32)
    b_dend = np.asarray(b_dend, np.float32)
    W_local = np.asarray(W_local, np.float32)
    b_local = np.asarray(b_local, np.float32)

    ximg, wimgs, wls = _host_images(x, W_dend, b_dend, W_local, b_local)
    nc = build_kernel()
    in_maps = [
        {"ximg": ximg, "wimg": wimgs[i], "wl": wls[i]}
        for i in range(N_CORES)
    ]
    res = run_bass_kernel_spmd(
        nc, in_maps, core_ids=list(range(N_CORES)), **_RUN_KWARGS)
    out = np.concatenate(
        [np.asarray(res.results[i]["out"], np.float32)
         for i in range(N_CORES)],
        axis=1,
    )
    kernel.last_results = res
    return out
